# revision 45
# baseline (speedup 1.0000x reference)
"""Trainium2 Bass kernel for nn_Attention_85907935855327.

Dense transformer attention block, B=128 S=196 D=512, H=8 heads (DK=32, DV=128),
BatchNorm (inference) after both projections, hard-swish between attention and
output projection, plus a "faithful to original" transpose-reshape quirk.

Strategy (8 cores, data-parallel over batch, 16 batches/core):
  Host:  fold BN into weights; permute W_qkv columns into [Wq | Wk] (head-major)
         and Wv (head-major); pre-transpose x -> xT [b, 512, 196].
  Device per batch pair (free-dim concat of 2 batches => N=392 >= 256 keeps
  fp32r matmuls at 1 cycle/row):
    qkT  [512, 392]  = Wqk^T @ xT            (fp32r)
    v    [196, 1024] = x @ Wv                (fp32r, lhsT = xT tiles)
    per head (bf16 attention core, fp32 PSUM accumulation):
      S^T  [196k, 196q] = k @ q^T            (lhsT = kT, rhs = qT, both native rows)
      E^T  = exp(scale * S^T)                (no max subtraction; logits |x| <= ~10)
      colsum[1, 392]  = ones^T @ E^T         (pair-concat, PE ones-matmul)
      r = 1/colsum (DVE);  R = GPSIMD partition_broadcast(r)
      U^T [128, 392] = v^T @ E^T             (unnormalized attention output^T)
      avT = (U^T * R + bv) -> hard-swish     (deferred softmax norm; the v-bias
                                              folds to a per-partition add because
                                              P @ (1 bv^T) = 1 bv^T after norm)
      write avT to DRAM M[b] rows h*128..h*128+128   (M is [1024, 196])
    The reference's transpose-reshape = reading M's flat buffer as H [196, 1024].
    H^T is needed for the output projection => PE transposes of bf16 H blocks
    (identity-matmul), with H preloaded right after the avT writes and the
    transposes + projection deferred one pair (software pipeline).
    outT [512, 392] = Wp^T @ H^T + b2        (bf16 matmul, fp32 accum)
  Host:  transpose outT back and assemble.

  Engine balance per pair (~31 us): PE ~24.5 us (QK/V/S/colsum/AV/transp/proj),
  DVE ~16 us (bias copies, t/a/av hswish steps, reciprocal), ACT ~17 us (exp,
  v copies), GPSIMD ~15 us (broadcast + 2 hswish steps), HWDGE ~10 DMAs.
"""

import numpy as np

B, S, D_IN = 128, 196, 512
H, DK, DV = 8, 32, 128
QKV_DIM = H * (2 * DK + DV)
PROJ_IN = H * DV  # 1024
EPS = 1e-3
SCALE = DK ** -0.5
NCORES = 8
BPC = B // NCORES          # batches per core
NPAIR = BPC // 2           # batch pairs per core
S2 = 2 * S                 # 392

_cache = {}


def _build():
    from contextlib import ExitStack
    import concourse.bass as bass
    import concourse.mybir as mybir
    import concourse.tile as tile
    from concourse import bacc
    from concourse.masks import make_identity

    F32 = mybir.dt.float32
    F32R = mybir.dt.float32r
    BF16 = mybir.dt.bfloat16
    AFT = mybir.ActivationFunctionType
    ALU = mybir.AluOpType

    nc = bacc.Bacc()

    xT_d = nc.declare_dram_parameter("xT", [BPC, D_IN, S], F32R, isOutput=False)
    wqk_d = nc.declare_dram_parameter("wqk", [D_IN, 512], F32R, isOutput=False)
    wv_d = nc.declare_dram_parameter("wv", [D_IN, PROJ_IN], F32R, isOutput=False)
    wp_d = nc.declare_dram_parameter("wp", [PROJ_IN, D_IN], BF16, isOutput=False)
    bqk_d = nc.declare_dram_parameter("bqk", [128, 4], F32, isOutput=False)
    bv_d = nc.declare_dram_parameter("bv", [128, H], F32, isOutput=False)
    b2_d = nc.declare_dram_parameter("b2", [128, 4], F32, isOutput=False)
    outT_d = nc.declare_dram_parameter("outT", [BPC, D_IN, S], F32, isOutput=True)
    import os as _os
    _DBG = bool(_os.environ.get("KDBG"))
    if _DBG:
        dbgM_d = nc.declare_dram_parameter("dbgM", [2, PROJ_IN + 64, S], mybir.dt.bfloat16, isOutput=True)
        dbgHT_d = nc.declare_dram_parameter("dbgHT", [128, 2, 208], mybir.dt.bfloat16, isOutput=True)
    _DBG2 = bool(_os.environ.get("KDBG2"))
    if _DBG2:
        dbgHT2_d = nc.declare_dram_parameter("dbgHT2", [8, 128, 2, 208], mybir.dt.bfloat16, isOutput=True)
        dbgOB_d = nc.declare_dram_parameter("dbgOB", [128, 4, S2], mybir.dt.float32, isOutput=True)
        dbgM2_d = nc.declare_dram_parameter("dbgM2", [2, PROJ_IN + 64, S], mybir.dt.bfloat16, isOutput=True)

    MT = [(0, 128), (128, 68)]  # seq m/k tile (offset, size)

    with tile.TileContext(nc) as tc, ExitStack() as ctx:
        wpool = ctx.enter_context(tc.tile_pool(name="w", bufs=1))
        xpool = ctx.enter_context(tc.tile_pool(name="x", bufs=2))
        qkpool = ctx.enter_context(tc.tile_pool(name="qk", bufs=2))
        vpool = ctx.enter_context(tc.tile_pool(name="v", bufs=2))
        epool = ctx.enter_context(tc.tile_pool(name="e", bufs=3))
        apool = ctx.enter_context(tc.tile_pool(name="a", bufs=3))
        hpool = ctx.enter_context(tc.tile_pool(name="h", bufs=2))
        opool = ctx.enter_context(tc.tile_pool(name="o", bufs=2))
        psum = ctx.enter_context(tc.tile_pool(name="ps", bufs=8, space="PSUM"))
        mdram = ctx.enter_context(tc.tile_pool(name="md", bufs=3, space="DRAM"))

        def load_xt(pair_idx):
            bb = 2 * pair_idx
            tiles = []
            for k in range(4):
                t = xpool.tile([128, 2, S], F32R, tag=f"xt{k}", bufs=3,
                               name=f"xt{pair_idx}_{k}")
                nc.sync.dma_start(
                    out=t,
                    in_=xT_d[bb:bb + 2, k * 128:(k + 1) * 128, :].rearrange(
                        "b p s -> p b s"),
                )
                tiles.append(t)
            return tiles

        # ---- persistent weights / constants ----
        wqk_sb = []
        wv_sb = []
        wp_sb = []
        for k in range(4):
            t = wpool.tile([128, 512], F32R, tag=f"wqk{k}")
            nc.sync.dma_start(out=t, in_=wqk_d[k * 128:(k + 1) * 128, :])
            wqk_sb.append(t)
        xt_next = load_xt(0)  # first pair's activations next in DMA queue
        for k in range(4):
            t = wpool.tile([128, PROJ_IN], F32R, tag=f"wv{k}")
            nc.sync.dma_start(out=t, in_=wv_d[k * 128:(k + 1) * 128, :])
            wv_sb.append(t)
        for k in range(8):
            t = wpool.tile([128, 512], BF16, tag=f"wp{k}")
            nc.sync.dma_start(out=t, in_=wp_d[k * 128:(k + 1) * 128, :])
            wp_sb.append(t)
        bqk_sb = wpool.tile([128, 4], F32, tag="bqk")
        nc.sync.dma_start(out=bqk_sb, in_=bqk_d[:, :])
        bv_sb = wpool.tile([128, H], F32, tag="bv")
        nc.sync.dma_start(out=bv_sb, in_=bv_d[:, :])
        b2_sb = wpool.tile([128, 4], F32, tag="b2")
        nc.sync.dma_start(out=b2_sb, in_=b2_d[:, :])

        ones_bf = wpool.tile([128, 1], BF16, tag="ones_bf")
        nc.vector.memset(ones_bf, 1.0)
        ident_f = wpool.tile([128, 128], F32, tag="ident_f")
        make_identity(nc, ident_f)
        ident = wpool.tile([128, 128], BF16, tag="ident")
        nc.vector.tensor_copy(ident, ident_f)

        pending_out = None  # deferred output stage of the previous pair

        def load_h_tiles(b0_, md_):
            """Emit H loads right after the avT writes: one full-width DMA per
            (bi, i-block) — 2KB contiguous per partition, minimal DMA count."""
            hs = []
            for bi in range(2):
                hview = md_[bi].rearrange("p s -> (p s)")[0:200704]                    .rearrange("(i j) -> i j", j=PROJ_IN)
                hb0 = hpool.tile([128, PROJ_IN], BF16, tag=f"hb0_{bi}",
                                 name=f"hb0_{b0_}_{bi}")
                nc.sync.dma_start(out=hb0, in_=hview[0:128, :])
                hb1 = hpool.tile([128, PROJ_IN], BF16, tag=f"hb1_{bi}",
                                 name=f"hb1_{b0_}_{bi}")
                nc.sync.dma_start(out=hb1[0:68], in_=hview[128:S, :])
                hs.append((hb0, hb1))
            return hs

        def emit_out_chunk(b0_, hs_, J2, ht_tiles):
            pts = [psum.tile([128, S2], BF16, tag="ps",
                             name=f"pt{b0_}_{J2}_{jj2}") for jj2 in range(2)]
            for bi in range(2):
                hb0, hb1 = hs_[bi]
                for jj in range(2):
                    c0 = J2 * 256 + jj * 128
                    nc.tensor.transpose(
                        pts[jj][:, bi * S:bi * S + 128],
                        hb0[:, c0:c0 + 128], ident)
                    nc.tensor.transpose(
                        pts[jj][:, bi * S + 128:bi * S + S],
                        hb1[0:68, c0:c0 + 128], ident[0:68, 0:68])
            for jj in range(2):
                ht = hpool.tile([128, S2], BF16, tag=f"ht{2 * J2 + jj}")
                nc.any.tensor_copy(ht, pts[jj])
                ht_tiles.append(ht)

        def run_output_stage(po_args, ht_tiles=None):
            (b0_, hs_) = po_args
            if ht_tiles is None:
                ht_tiles = []
            for J2 in range(4 - len(ht_tiles) // 2, 4):
                pass  # chunks already emitted by caller
            for J2 in range(len(ht_tiles) // 2, 4):
                emit_out_chunk(b0_, hs_, J2, ht_tiles)

            ob = opool.tile([128, 4, S2], F32, tag="ob", bufs=2,
                            name=f"ob{b0_}")
            for m in range(4):
                po = psum.tile([128, S2], F32, tag="ps")
                for kk in range(8):
                    nc.tensor.matmul(
                        po, wp_sb[kk][:, m * 128:(m + 1) * 128], ht_tiles[kk],
                        start=(kk == 0), stop=(kk == 7),
                    )
                nc.vector.tensor_scalar_add(ob[:, m, :], po, b2_sb[:, m:m + 1])
            for bi in range(2):
                nc.sync.dma_start(
                    out=outT_d[b0_ + bi].rearrange("(m p) s -> p m s", p=128),
                    in_=ob[:, :, bi * S:(bi + 1) * S])
            if _DBG2 and b0_ == 0:
                for bi in range(2):
                    nc.sync.dma_start(out=dbgM2_d[bi], in_=md_[bi][:, :])
                nc.sync.dma_start(out=dbgOB_d[:, :, :], in_=ob[:, :, :])

        for pr in range(NPAIR):
            b0 = 2 * pr
            xt = xt_next
            if pr + 1 < NPAIR:
                xt_next = load_xt(pr + 1)

            # ---- qkT: rows 0:256 = q heads, 256:512 = k heads ----
            qk_sb = []
            for m in range(4):
                ps = psum.tile([128, S2], F32, tag="ps")
                for k in range(4):
                    nc.tensor.matmul(
                        ps, wqk_sb[k][:, m * 128:(m + 1) * 128], xt[k],
                        start=(k == 0), stop=(k == 3),
                    )
                qt = qkpool.tile([128, 2, S], BF16, tag=f"qk{m}", bufs=3)
                nc.vector.tensor_scalar_add(qt, ps, bqk_sb[:, m:m + 1])
                qk_sb.append(qt)

            # ---- v natural per batch: [196, 1024] bf16, 2 seq tiles ----
            v_sb = [[None, None], [None, None]]
            for bi in range(2):
                for m2, (off, sz) in enumerate(MT):
                    vt = vpool.tile([128, PROJ_IN], BF16, tag=f"v{bi}{m2}", bufs=3)
                    for n in range(2):
                        ps = psum.tile([128, 512], F32, tag="ps")
                        for k in range(4):
                            nc.tensor.matmul(
                                ps[:sz], xt[k][:, bi, off:off + sz],
                                wv_sb[k][:, n * 512:(n + 1) * 512],
                                start=(k == 0), stop=(k == 3),
                            )
                        nc.any.tensor_copy(vt[:sz, n * 512:(n + 1) * 512], ps[:sz])
                    v_sb[bi][m2] = vt

            # ---- M scratch (one per batch): [1024, 196] = stacked avT ----
            md = [mdram.tile([PROJ_IN + 64, S], BF16, tag=f"M{bi}", name=f"md{pr}_{bi}") for bi in range(2)]

            # ---- attention, stage-grouped across heads for overlap ----
            # Phase A: S^T matmuls (pair psum, bi halves) + exp -> et tiles
            ets = []
            for h in range(8):
                g, hh = divmod(h, 4)
                q_tile, k_tile = qk_sb[g], qk_sb[2 + g]
                r0 = 32 * hh
                et = [epool.tile([128, 2, S], BF16, tag=f"et{h}_{mt}",
                                  name=f"et{pr}_{h}_{mt}", bufs=2)
                      for mt in range(2)]
                for mt, (off, sz) in enumerate(MT):
                    ps = psum.tile([128, S2], F32, tag="ps")
                    for bi in range(2):
                        nc.tensor.matmul(
                            ps[:sz, bi * S:(bi + 1) * S],
                            k_tile[r0:r0 + 32, bi, off:off + sz],
                            q_tile[r0:r0 + 32, bi, :],
                            start=True, stop=True,
                            tile_position=(r0, 0),
                        )
                    nc.scalar.activation(et[mt][:sz], ps[:sz], AFT.Exp,
                                         scale=SCALE)
                ets.append(et)

            # Phase B: colsums (PE ones-matmul) + reciprocals (DVE)
            rws_all = []
            for h in range(8):
                et = ets[h]
                pc = psum.tile([1, S2], F32, tag="ps")
                nc.tensor.matmul(pc, ones_bf, et[0], start=True, stop=False)
                nc.tensor.matmul(pc, ones_bf[0:68], et[1][0:68],
                                 start=False, stop=True)
                rws = apool.tile([1, S2], F32R, tag=f"rws{h % 2}", bufs=2,
                                 name=f"rws{pr}_{h}")
                with nc.allow_low_precision(reason="fp22 fine for softmax norm"):
                    nc.vector.reciprocal(rws, pc)
                rws_all.append(rws)

            # Phase C: all R broadcasts up-front so GPSIMD never gates PSUM drain
            R_all = []
            for h in range(8):
                R_h = apool.tile([128, S2], F32R, tag=f"R{h}", bufs=1,
                                 name=f"R{pr}_{h}")
                nc.gpsimd.partition_broadcast(R_h, rws_all[h][0:1, :])
                R_all.append(R_h)

            # Per head: U^T + hswish chain; M write merged below.
            # The previous pair's transpose chunks slot in every other head to
            # fill PE bubbles while the hswish chain drains U^T psum slots.
            out_ht_tiles = []
            av_all = apool.tile([128, 8, S2], BF16, tag="av_all", bufs=2,
                                name=f"av_all{pr}")
            for h in range(8):
                if pending_out is not None and h >= 4 and h % 2 == 0:
                    emit_out_chunk(pending_out[0], pending_out[1],
                                   (h - 4) // 2, out_ht_tiles)
                et = ets[h]
                R_h = R_all[h]
                pu = psum.tile([128, S2], F32, tag="ps")
                for bi in range(2):
                    nc.tensor.matmul(
                        pu[:, bi * S:(bi + 1) * S],
                        v_sb[bi][0][:, h * 128:(h + 1) * 128],
                        et[0][:, bi, :], start=True, stop=False)
                    nc.tensor.matmul(
                        pu[:, bi * S:(bi + 1) * S],
                        v_sb[bi][1][0:68, h * 128:(h + 1) * 128],
                        et[1][0:68, bi, :], start=False, stop=True)
                # avT = hswish(U^T * R + bv); hswish(a) = a*min(relu(a/6+.5),1)
                t_ = apool.tile([128, S2], F32, tag="t", bufs=3)
                nc.vector.tensor_mul(t_, pu, R_h)
                a_ = apool.tile([128, S2], F32, tag="a", bufs=3)
                nc.vector.tensor_scalar_add(a_, t_, bv_sb[:, h:h + 1])
                u_ = apool.tile([128, S2], F32, tag="u", bufs=3)
                nc.gpsimd.tensor_scalar(u_, a_, 3.0, 0.0, ALU.add, ALU.max)
                w_ = apool.tile([128, S2], F32, tag="wm", bufs=3)
                nc.gpsimd.tensor_scalar(w_, u_, 6.0, 1.0 / 6.0, ALU.min,
                                        ALU.mult)
                nc.vector.tensor_mul(av_all[:, h, :], a_, w_)
            av_writes = []
            for bi in range(2):
                wr = nc.sync.dma_start(
                    out=md[bi][0:PROJ_IN, :].rearrange("(h p) s -> p h s", p=128),
                    in_=av_all[:, :, bi * S:(bi + 1) * S])
                av_writes.append(wr)

            if _DBG and pr == 0:
                for bi in range(2):
                    nc.sync.dma_start(out=dbgM_d[bi], in_=md[bi][:, :])

            hs_now = load_h_tiles(b0, md)

            # ---- output stage: finish the PREVIOUS pair's (chunks 2,3 + proj)
            if pending_out is not None:
                run_output_stage(pending_out, out_ht_tiles)
            pending_out = (b0, hs_now)

        run_output_stage(pending_out)

    nc.compile()
    return nc


def _get_nc():
    if "nc" not in _cache:
        _cache["nc"] = _build()
    return _cache["nc"]


def _prep(inputs):
    """Host-side BN folding / weight permutation / x transpose."""
    f = np.float32
    gamma1, beta1 = inputs["gamma1"].astype(f), inputs["beta1"].astype(f)
    mean1, var1 = inputs["mean1"].astype(f), inputs["var1"].astype(f)
    gamma2, beta2 = inputs["gamma2"].astype(f), inputs["beta2"].astype(f)
    mean2, var2 = inputs["mean2"].astype(f), inputs["var2"].astype(f)

    a1 = gamma1 / np.sqrt(var1 + EPS)
    c1 = beta1 - mean1 * a1
    W1 = inputs["W_qkv"].astype(f) * a1[None, :]
    B1 = inputs["b_qkv"].astype(f) * a1 + c1
    a2 = gamma2 / np.sqrt(var2 + EPS)
    c2 = beta2 - mean2 * a2
    W2 = np.ascontiguousarray(inputs["W_proj"].astype(f) * a2[None, :])
    B2 = inputs["b_proj"].astype(f) * a2 + c2

    W1h = W1.reshape(D_IN, H, 2 * DK + DV)
    B1h = B1.reshape(H, 2 * DK + DV)
    Wq = W1h[:, :, 0:DK].reshape(D_IN, H * DK)
    Wk = W1h[:, :, DK:2 * DK].reshape(D_IN, H * DK)
    Wv = np.ascontiguousarray(W1h[:, :, 2 * DK:].reshape(D_IN, PROJ_IN))
    Wqk = np.ascontiguousarray(np.concatenate([Wq, Wk], axis=1))
    bqk = np.concatenate([B1h[:, 0:DK].reshape(-1), B1h[:, DK:2 * DK].reshape(-1)])
    bv = B1h[:, 2 * DK:].reshape(-1)  # [1024] head-major

    x = inputs["x"].astype(f)
    xT = np.ascontiguousarray(x.transpose(0, 2, 1))  # [B, 512, 196]

    bqk_r = np.ascontiguousarray(bqk.reshape(4, 128).T)   # [128, 4]
    b2_r = np.ascontiguousarray(B2.reshape(4, 128).T)     # [128, 4]
    bv_r = np.ascontiguousarray(bv.reshape(H, DV).T)      # [128, 8]
    return xT, Wqk, Wv, W2, bqk_r, bv_r, b2_r


def kernel(**inputs) -> np.ndarray:
    import ml_dtypes
    from concourse.bass_utils import run_bass_kernel_spmd

    xT, Wqk, Wv, W2, bqk_r, bv_r, b2_r = _prep(inputs)
    nc = _get_nc()

    W2b = W2.astype(ml_dtypes.bfloat16)
    in_maps = []
    for c in range(NCORES):
        in_maps.append({
            "xT": np.ascontiguousarray(xT[c * BPC:(c + 1) * BPC]),
            "wqk": Wqk, "wv": Wv, "wp": W2b,
            "bqk": bqk_r, "bv": bv_r, "b2": b2_r,
        })
    res = run_bass_kernel_spmd(nc, in_maps, list(range(NCORES)))
    outT = np.concatenate([res.results[c]["outT"] for c in range(NCORES)], axis=0)
    out = np.ascontiguousarray(outT.transpose(0, 2, 1)).astype(np.float32)
    return out


# revision 46
# speedup vs baseline: 1.0133x; 1.0133x over previous
"""Trainium2 Bass kernel for nn_Attention_85907935855327.

Dense transformer attention block, B=128 S=196 D=512, H=8 heads (DK=32, DV=128),
BatchNorm (inference) after both projections, hard-swish between attention and
output projection, plus a "faithful to original" transpose-reshape quirk.

Strategy (8 cores, data-parallel over batch, 16 batches/core):
  Host:  fold BN into weights; permute W_qkv columns into [Wq | Wk] (head-major)
         and Wv (head-major); pre-transpose x -> xT [b, 512, 196].
  Device per batch pair (free-dim concat of 2 batches => N=392 >= 256 keeps
  fp32r matmuls at 1 cycle/row):
    qkT  [512, 392]  = Wqk^T @ xT            (fp32r)
    v    [196, 1024] = x @ Wv                (fp32r, lhsT = xT tiles)
    per head (bf16 attention core, fp32 PSUM accumulation):
      S^T  [196k, 196q] = k @ q^T            (lhsT = kT, rhs = qT, both native rows)
      E^T  = exp(scale * S^T)                (no max subtraction; logits |x| <= ~10)
      colsum[1, 392]  = ones^T @ E^T         (pair-concat, PE ones-matmul)
      r = 1/colsum (DVE);  R = GPSIMD partition_broadcast(r)
      U^T [128, 392] = v^T @ E^T             (unnormalized attention output^T)
      avT = (U^T * R + bv) -> hard-swish     (deferred softmax norm; the v-bias
                                              folds to a per-partition add because
                                              P @ (1 bv^T) = 1 bv^T after norm)
      write avT to DRAM M[b] rows h*128..h*128+128   (M is [1024, 196])
    The reference's transpose-reshape = reading M's flat buffer as H [196, 1024].
    H^T is needed for the output projection => PE transposes of bf16 H blocks
    (identity-matmul), with H preloaded right after the avT writes and the
    transposes + projection deferred one pair (software pipeline).
    outT [512, 392] = Wp^T @ H^T + b2        (bf16 matmul, fp32 accum)
  Host:  transpose outT back and assemble.

  Engine balance per pair (~31 us): PE ~24.5 us (QK/V/S/colsum/AV/transp/proj),
  DVE ~16 us (bias copies, t/a/av hswish steps, reciprocal), ACT ~17 us (exp,
  v copies), GPSIMD ~15 us (broadcast + 2 hswish steps), HWDGE ~10 DMAs.
"""

import numpy as np

B, S, D_IN = 128, 196, 512
H, DK, DV = 8, 32, 128
QKV_DIM = H * (2 * DK + DV)
PROJ_IN = H * DV  # 1024
EPS = 1e-3
SCALE = DK ** -0.5
NCORES = 8
BPC = B // NCORES          # batches per core
NPAIR = BPC // 2           # batch pairs per core
S2 = 2 * S                 # 392

_cache = {}


def _build():
    from contextlib import ExitStack
    import concourse.bass as bass
    import concourse.mybir as mybir
    import concourse.tile as tile
    from concourse import bacc
    from concourse.masks import make_identity

    F32 = mybir.dt.float32
    F32R = mybir.dt.float32r
    BF16 = mybir.dt.bfloat16
    AFT = mybir.ActivationFunctionType
    ALU = mybir.AluOpType

    nc = bacc.Bacc()

    xT_d = nc.declare_dram_parameter("xT", [BPC, D_IN, S], F32R, isOutput=False)
    wqk_d = nc.declare_dram_parameter("wqk", [D_IN, 512], F32R, isOutput=False)
    wv_d = nc.declare_dram_parameter("wv", [D_IN, PROJ_IN], F32R, isOutput=False)
    wp_d = nc.declare_dram_parameter("wp", [PROJ_IN, D_IN], BF16, isOutput=False)
    bqk_d = nc.declare_dram_parameter("bqk", [128, 4], F32, isOutput=False)
    bv_d = nc.declare_dram_parameter("bv", [128, H], F32, isOutput=False)
    b2_d = nc.declare_dram_parameter("b2", [128, 4], F32, isOutput=False)
    outT_d = nc.declare_dram_parameter("outT", [BPC, D_IN, S], F32, isOutput=True)
    import os as _os
    _DBG = bool(_os.environ.get("KDBG"))
    if _DBG:
        dbgM_d = nc.declare_dram_parameter("dbgM", [2, PROJ_IN + 64, S], mybir.dt.bfloat16, isOutput=True)
        dbgHT_d = nc.declare_dram_parameter("dbgHT", [128, 2, 208], mybir.dt.bfloat16, isOutput=True)
    _DBG2 = bool(_os.environ.get("KDBG2"))
    if _DBG2:
        dbgHT2_d = nc.declare_dram_parameter("dbgHT2", [8, 128, 2, 208], mybir.dt.bfloat16, isOutput=True)
        dbgOB_d = nc.declare_dram_parameter("dbgOB", [128, 4, S2], mybir.dt.float32, isOutput=True)
        dbgM2_d = nc.declare_dram_parameter("dbgM2", [2, PROJ_IN + 64, S], mybir.dt.bfloat16, isOutput=True)

    MT = [(0, 128), (128, 68)]  # seq m/k tile (offset, size)

    with tile.TileContext(nc) as tc, ExitStack() as ctx:
        wpool = ctx.enter_context(tc.tile_pool(name="w", bufs=1))
        xpool = ctx.enter_context(tc.tile_pool(name="x", bufs=2))
        qkpool = ctx.enter_context(tc.tile_pool(name="qk", bufs=2))
        vpool = ctx.enter_context(tc.tile_pool(name="v", bufs=2))
        epool = ctx.enter_context(tc.tile_pool(name="e", bufs=3))
        apool = ctx.enter_context(tc.tile_pool(name="a", bufs=3))
        hpool = ctx.enter_context(tc.tile_pool(name="h", bufs=2))
        opool = ctx.enter_context(tc.tile_pool(name="o", bufs=2))
        psum = ctx.enter_context(tc.tile_pool(name="ps", bufs=8, space="PSUM"))
        mdram = ctx.enter_context(tc.tile_pool(name="md", bufs=3, space="DRAM"))

        def load_xt(pair_idx):
            bb = 2 * pair_idx
            tiles = []
            for k in range(4):
                t = xpool.tile([128, 2, S], F32R, tag=f"xt{k}", bufs=2,
                               name=f"xt{pair_idx}_{k}")
                nc.sync.dma_start(
                    out=t,
                    in_=xT_d[bb:bb + 2, k * 128:(k + 1) * 128, :].rearrange(
                        "b p s -> p b s"),
                )
                tiles.append(t)
            return tiles

        # ---- persistent weights / constants ----
        wqk_sb = []
        wv_sb = []
        wp_sb = []
        for k in range(4):
            t = wpool.tile([128, 512], F32R, tag=f"wqk{k}")
            nc.sync.dma_start(out=t, in_=wqk_d[k * 128:(k + 1) * 128, :])
            wqk_sb.append(t)
        xt_next = load_xt(0)  # first pair's activations next in DMA queue
        for k in range(4):
            t = wpool.tile([128, PROJ_IN], F32R, tag=f"wv{k}")
            nc.sync.dma_start(out=t, in_=wv_d[k * 128:(k + 1) * 128, :])
            wv_sb.append(t)
        for k in range(8):
            t = wpool.tile([128, 512], BF16, tag=f"wp{k}")
            nc.sync.dma_start(out=t, in_=wp_d[k * 128:(k + 1) * 128, :])
            wp_sb.append(t)
        bqk_sb = wpool.tile([128, 4], F32, tag="bqk")
        nc.sync.dma_start(out=bqk_sb, in_=bqk_d[:, :])
        bv_sb = wpool.tile([128, H], F32, tag="bv")
        nc.sync.dma_start(out=bv_sb, in_=bv_d[:, :])
        b2_sb = wpool.tile([128, 4], F32, tag="b2")
        nc.sync.dma_start(out=b2_sb, in_=b2_d[:, :])

        ones_bf = wpool.tile([128, 1], BF16, tag="ones_bf")
        nc.vector.memset(ones_bf, 1.0)
        ident_f = wpool.tile([128, 128], F32, tag="ident_f")
        make_identity(nc, ident_f)
        ident = wpool.tile([128, 128], BF16, tag="ident")
        nc.vector.tensor_copy(ident, ident_f)

        pending_out = None  # deferred output stage of the previous pair

        def load_h_tiles(b0_, md_):
            """Emit H loads right after the avT writes: one full-width DMA per
            (bi, i-block) — 2KB contiguous per partition, minimal DMA count."""
            hs = []
            for bi in range(2):
                hview = md_[bi].rearrange("p s -> (p s)")[0:200704]                    .rearrange("(i j) -> i j", j=PROJ_IN)
                hb0 = hpool.tile([128, PROJ_IN], BF16, tag=f"hb0_{bi}",
                                 name=f"hb0_{b0_}_{bi}")
                nc.sync.dma_start(out=hb0, in_=hview[0:128, :])
                hb1 = hpool.tile([128, PROJ_IN], BF16, tag=f"hb1_{bi}",
                                 name=f"hb1_{b0_}_{bi}")
                nc.sync.dma_start(out=hb1[0:68], in_=hview[128:S, :])
                hs.append((hb0, hb1))
            return hs

        def emit_out_chunk(b0_, hs_, J2, ht_tiles):
            pts = [psum.tile([128, S2], BF16, tag="ps",
                             name=f"pt{b0_}_{J2}_{jj2}") for jj2 in range(2)]
            for bi in range(2):
                hb0, hb1 = hs_[bi]
                for jj in range(2):
                    c0 = J2 * 256 + jj * 128
                    nc.tensor.transpose(
                        pts[jj][:, bi * S:bi * S + 128],
                        hb0[:, c0:c0 + 128], ident)
                    nc.tensor.transpose(
                        pts[jj][:, bi * S + 128:bi * S + S],
                        hb1[0:68, c0:c0 + 128], ident[0:68, 0:68])
            for jj in range(2):
                ht = hpool.tile([128, S2], BF16, tag=f"ht{2 * J2 + jj}")
                nc.any.tensor_copy(ht, pts[jj])
                ht_tiles.append(ht)

        def run_output_stage(po_args, ht_tiles=None):
            (b0_, hs_) = po_args
            if ht_tiles is None:
                ht_tiles = []
            for J2 in range(4 - len(ht_tiles) // 2, 4):
                pass  # chunks already emitted by caller
            for J2 in range(len(ht_tiles) // 2, 4):
                emit_out_chunk(b0_, hs_, J2, ht_tiles)

            ob = opool.tile([128, 4, S2], F32, tag="ob", bufs=2,
                            name=f"ob{b0_}")
            for m in range(4):
                po = psum.tile([128, S2], F32, tag="ps")
                for kk in range(8):
                    nc.tensor.matmul(
                        po, wp_sb[kk][:, m * 128:(m + 1) * 128], ht_tiles[kk],
                        start=(kk == 0), stop=(kk == 7),
                    )
                nc.vector.tensor_scalar_add(ob[:, m, :], po, b2_sb[:, m:m + 1])
            for bi in range(2):
                nc.sync.dma_start(
                    out=outT_d[b0_ + bi].rearrange("(m p) s -> p m s", p=128),
                    in_=ob[:, :, bi * S:(bi + 1) * S])
            if _DBG2 and b0_ == 0:
                for bi in range(2):
                    nc.sync.dma_start(out=dbgM2_d[bi], in_=md_[bi][:, :])
                nc.sync.dma_start(out=dbgOB_d[:, :, :], in_=ob[:, :, :])

        for pr in range(NPAIR):
            b0 = 2 * pr
            xt = xt_next
            if pr + 1 < NPAIR:
                xt_next = load_xt(pr + 1)

            # ---- qkT: rows 0:256 = q heads, 256:512 = k heads ----
            qk_sb = []
            for m in range(4):
                ps = psum.tile([128, S2], F32, tag="ps")
                for k in range(4):
                    nc.tensor.matmul(
                        ps, wqk_sb[k][:, m * 128:(m + 1) * 128], xt[k],
                        start=(k == 0), stop=(k == 3),
                    )
                qt = qkpool.tile([128, 2, S], BF16, tag=f"qk{m}", bufs=3)
                nc.vector.tensor_scalar_add(qt, ps, bqk_sb[:, m:m + 1])
                qk_sb.append(qt)

            # ---- v natural per batch: [196, 1024] bf16, 2 seq tiles ----
            v_sb = [[None, None], [None, None]]
            for bi in range(2):
                for m2, (off, sz) in enumerate(MT):
                    vt = vpool.tile([128, PROJ_IN], BF16, tag=f"v{bi}{m2}", bufs=3)
                    for n in range(2):
                        ps = psum.tile([128, 512], F32, tag="ps")
                        for k in range(4):
                            nc.tensor.matmul(
                                ps[:sz], xt[k][:, bi, off:off + sz],
                                wv_sb[k][:, n * 512:(n + 1) * 512],
                                start=(k == 0), stop=(k == 3),
                            )
                        nc.any.tensor_copy(vt[:sz, n * 512:(n + 1) * 512], ps[:sz])
                    v_sb[bi][m2] = vt

            # ---- M scratch (one per batch): [1024, 196] = stacked avT ----
            md = [mdram.tile([PROJ_IN + 64, S], BF16, tag=f"M{bi}", name=f"md{pr}_{bi}") for bi in range(2)]

            # ---- attention, stage-grouped across heads for overlap ----
            # Phase A: S^T matmuls (pair psum, bi halves) + exp -> et tiles
            ets = []
            for h in range(8):
                g, hh = divmod(h, 4)
                q_tile, k_tile = qk_sb[g], qk_sb[2 + g]
                r0 = 32 * hh
                et = [epool.tile([128, 2, S], BF16, tag=f"et{h}_{mt}",
                                  name=f"et{pr}_{h}_{mt}", bufs=2)
                      for mt in range(2)]
                for mt, (off, sz) in enumerate(MT):
                    ps = psum.tile([128, S2], F32, tag="ps")
                    for bi in range(2):
                        nc.tensor.matmul(
                            ps[:sz, bi * S:(bi + 1) * S],
                            k_tile[r0:r0 + 32, bi, off:off + sz],
                            q_tile[r0:r0 + 32, bi, :],
                            start=True, stop=True,
                            tile_position=(r0, 0),
                        )
                    nc.scalar.activation(et[mt][:sz], ps[:sz], AFT.Exp,
                                         scale=SCALE)
                ets.append(et)

            # Phase B: colsums (PE ones-matmul) + reciprocals (DVE)
            rws_all = []
            for h in range(8):
                et = ets[h]
                pc = psum.tile([1, S2], F32, tag="ps")
                nc.tensor.matmul(pc, ones_bf, et[0], start=True, stop=False)
                nc.tensor.matmul(pc, ones_bf[0:68], et[1][0:68],
                                 start=False, stop=True)
                rws = apool.tile([1, S2], F32R, tag=f"rws{h % 2}", bufs=2,
                                 name=f"rws{pr}_{h}")
                with nc.allow_low_precision(reason="fp22 fine for softmax norm"):
                    nc.vector.reciprocal(rws, pc)
                rws_all.append(rws)

            # Phase C: all R broadcasts up-front so GPSIMD never gates PSUM drain
            R_all = []
            for h in range(8):
                R_h = apool.tile([128, S2], F32R, tag=f"R{h}", bufs=1,
                                 name=f"R{pr}_{h}")
                nc.gpsimd.partition_broadcast(R_h, rws_all[h][0:1, :])
                R_all.append(R_h)

            # Per head: U^T + hswish chain; M write merged below.
            # The previous pair's transpose chunks slot in every other head to
            # fill PE bubbles while the hswish chain drains U^T psum slots.
            out_ht_tiles = []
            av_all = apool.tile([128, 8, S2], BF16, tag="av_all", bufs=2,
                                name=f"av_all{pr}")
            def chain_tail(h, t_):
                # hswish tail: a = t + bv; avT = a*min(relu((a+3))/6,1)
                a_ = apool.tile([128, S2], F32, tag="a", bufs=3,
                                name=f"a{pr}_{h}")
                nc.vector.tensor_scalar_add(a_, t_, bv_sb[:, h:h + 1])
                u_ = apool.tile([128, S2], F32, tag="u", bufs=3,
                                name=f"u{pr}_{h}")
                nc.gpsimd.tensor_scalar(u_, a_, 3.0, 0.0, ALU.add, ALU.max)
                w_ = apool.tile([128, S2], F32, tag="wm", bufs=3,
                                name=f"w{pr}_{h}")
                nc.gpsimd.tensor_scalar(w_, u_, 6.0, 1.0 / 6.0, ALU.min,
                                        ALU.mult)
                nc.vector.tensor_mul(av_all[:, h, :], a_, w_)

            # Two waves of 4 heads: AV matmuls + t_ (frees the U^T psum fast)
            # first, hswish tails after — keeps PE's psum slots draining at
            # DVE t_ rate instead of the full cross-engine chain rate.
            t_held = {}
            for wave in range(2):
                for h in range(4 * wave, 4 * wave + 4):
                    if pending_out is not None and h >= 4 and h % 2 == 0:
                        emit_out_chunk(pending_out[0], pending_out[1],
                                       (h - 4) // 2, out_ht_tiles)
                    et = ets[h]
                    R_h = R_all[h]
                    pu = psum.tile([128, S2], F32, tag="ps")
                    for bi in range(2):
                        nc.tensor.matmul(
                            pu[:, bi * S:(bi + 1) * S],
                            v_sb[bi][0][:, h * 128:(h + 1) * 128],
                            et[0][:, bi, :], start=True, stop=False)
                        nc.tensor.matmul(
                            pu[:, bi * S:(bi + 1) * S],
                            v_sb[bi][1][0:68, h * 128:(h + 1) * 128],
                            et[1][0:68, bi, :], start=False, stop=True)
                    t_ = apool.tile([128, S2], F32, tag=f"t{h % 4}", bufs=2,
                                    name=f"t{pr}_{h}")
                    nc.vector.tensor_mul(t_, pu, R_h)
                    t_held[h] = t_
                for h in range(4 * wave, 4 * wave + 4):
                    chain_tail(h, t_held.pop(h))
            av_writes = []
            for bi in range(2):
                wr = nc.sync.dma_start(
                    out=md[bi][0:PROJ_IN, :].rearrange("(h p) s -> p h s", p=128),
                    in_=av_all[:, :, bi * S:(bi + 1) * S])
                av_writes.append(wr)

            if _DBG and pr == 0:
                for bi in range(2):
                    nc.sync.dma_start(out=dbgM_d[bi], in_=md[bi][:, :])

            hs_now = load_h_tiles(b0, md)

            # ---- output stage: finish the PREVIOUS pair's (chunks 2,3 + proj)
            if pending_out is not None:
                run_output_stage(pending_out, out_ht_tiles)
            pending_out = (b0, hs_now)

        run_output_stage(pending_out)

    nc.compile()
    return nc


def _get_nc():
    if "nc" not in _cache:
        _cache["nc"] = _build()
    return _cache["nc"]


def _prep(inputs):
    """Host-side BN folding / weight permutation / x transpose."""
    f = np.float32
    gamma1, beta1 = inputs["gamma1"].astype(f), inputs["beta1"].astype(f)
    mean1, var1 = inputs["mean1"].astype(f), inputs["var1"].astype(f)
    gamma2, beta2 = inputs["gamma2"].astype(f), inputs["beta2"].astype(f)
    mean2, var2 = inputs["mean2"].astype(f), inputs["var2"].astype(f)

    a1 = gamma1 / np.sqrt(var1 + EPS)
    c1 = beta1 - mean1 * a1
    W1 = inputs["W_qkv"].astype(f) * a1[None, :]
    B1 = inputs["b_qkv"].astype(f) * a1 + c1
    a2 = gamma2 / np.sqrt(var2 + EPS)
    c2 = beta2 - mean2 * a2
    W2 = np.ascontiguousarray(inputs["W_proj"].astype(f) * a2[None, :])
    B2 = inputs["b_proj"].astype(f) * a2 + c2

    W1h = W1.reshape(D_IN, H, 2 * DK + DV)
    B1h = B1.reshape(H, 2 * DK + DV)
    Wq = W1h[:, :, 0:DK].reshape(D_IN, H * DK)
    Wk = W1h[:, :, DK:2 * DK].reshape(D_IN, H * DK)
    Wv = np.ascontiguousarray(W1h[:, :, 2 * DK:].reshape(D_IN, PROJ_IN))
    Wqk = np.ascontiguousarray(np.concatenate([Wq, Wk], axis=1))
    bqk = np.concatenate([B1h[:, 0:DK].reshape(-1), B1h[:, DK:2 * DK].reshape(-1)])
    bv = B1h[:, 2 * DK:].reshape(-1)  # [1024] head-major

    x = inputs["x"].astype(f)
    xT = np.ascontiguousarray(x.transpose(0, 2, 1))  # [B, 512, 196]

    bqk_r = np.ascontiguousarray(bqk.reshape(4, 128).T)   # [128, 4]
    b2_r = np.ascontiguousarray(B2.reshape(4, 128).T)     # [128, 4]
    bv_r = np.ascontiguousarray(bv.reshape(H, DV).T)      # [128, 8]
    return xT, Wqk, Wv, W2, bqk_r, bv_r, b2_r


def kernel(**inputs) -> np.ndarray:
    import ml_dtypes
    from concourse.bass_utils import run_bass_kernel_spmd

    xT, Wqk, Wv, W2, bqk_r, bv_r, b2_r = _prep(inputs)
    nc = _get_nc()

    W2b = W2.astype(ml_dtypes.bfloat16)
    in_maps = []
    for c in range(NCORES):
        in_maps.append({
            "xT": np.ascontiguousarray(xT[c * BPC:(c + 1) * BPC]),
            "wqk": Wqk, "wv": Wv, "wp": W2b,
            "bqk": bqk_r, "bv": bv_r, "b2": b2_r,
        })
    res = run_bass_kernel_spmd(nc, in_maps, list(range(NCORES)))
    outT = np.concatenate([res.results[c]["outT"] for c in range(NCORES)], axis=0)
    out = np.ascontiguousarray(outT.transpose(0, 2, 1)).astype(np.float32)
    return out


# revision 49
# speedup vs baseline: 1.0664x; 1.0524x over previous
"""Trainium2 Bass kernel for nn_Attention_85907935855327.

Dense transformer attention block, B=128 S=196 D=512, H=8 heads (DK=32, DV=128),
BatchNorm (inference) after both projections, hard-swish between attention and
output projection, plus a "faithful to original" transpose-reshape quirk.

Strategy (8 cores, data-parallel over batch, 16 batches/core):
  Host:  fold BN into weights; permute W_qkv columns into [Wq | Wk] (head-major)
         and Wv (head-major); pre-transpose x -> xT [b, 512, 196].
  Device per batch pair (free-dim concat of 2 batches => N=392 >= 256 keeps
  fp32r matmuls at 1 cycle/row):
    qkT  [512, 392]  = Wqk^T @ xT            (fp32r)
    v    [196, 1024] = x @ Wv                (fp32r, lhsT = xT tiles)
    per head (bf16 attention core, fp32 PSUM accumulation):
      S^T  [196k, 196q] = k @ q^T            (lhsT = kT, rhs = qT, both native rows)
      E^T  = exp(scale * S^T)                (no max subtraction; logits |x| <= ~10)
      colsum[1, 392]  = ones^T @ E^T         (pair-concat, PE ones-matmul)
      r = 1/colsum (DVE);  R = GPSIMD partition_broadcast(r)
      U^T [128, 392] = v^T @ E^T             (unnormalized attention output^T)
      avT = (U^T * R + bv) -> hard-swish     (deferred softmax norm; the v-bias
                                              folds to a per-partition add because
                                              P @ (1 bv^T) = 1 bv^T after norm)
      write avT to DRAM M[b] rows h*128..h*128+128   (M is [1024, 196])
    The reference's transpose-reshape = reading M's flat buffer as H [196, 1024].
    H^T is needed for the output projection => PE transposes of bf16 H blocks
    (identity-matmul), with H preloaded right after the avT writes and the
    transposes + projection deferred one pair (software pipeline).
    outT [512, 392] = Wp^T @ H^T + b2        (bf16 matmul, fp32 accum)
  Host:  transpose outT back and assemble.

  Engine balance per pair (~31 us): PE ~24.5 us (QK/V/S/colsum/AV/transp/proj),
  DVE ~16 us (bias copies, t/a/av hswish steps, reciprocal), ACT ~17 us (exp,
  v copies), GPSIMD ~15 us (broadcast + 2 hswish steps), HWDGE ~10 DMAs.
"""

import numpy as np

B, S, D_IN = 128, 196, 512
H, DK, DV = 8, 32, 128
QKV_DIM = H * (2 * DK + DV)
PROJ_IN = H * DV  # 1024
EPS = 1e-3
SCALE = DK ** -0.5
NCORES = 8
BPC = B // NCORES          # batches per core
NPAIR = BPC // 2           # batch pairs per core
S2 = 2 * S                 # 392

_cache = {}


def _build():
    from contextlib import ExitStack
    import concourse.bass as bass
    import concourse.mybir as mybir
    import concourse.tile as tile
    from concourse import bacc
    from concourse.masks import make_identity

    F32 = mybir.dt.float32
    F32R = mybir.dt.float32r
    BF16 = mybir.dt.bfloat16
    AFT = mybir.ActivationFunctionType
    ALU = mybir.AluOpType

    nc = bacc.Bacc()

    xT_d = nc.declare_dram_parameter("xT", [BPC, D_IN, S], F32R, isOutput=False)
    wqk_d = nc.declare_dram_parameter("wqk", [D_IN, 512], F32R, isOutput=False)
    wv_d = nc.declare_dram_parameter("wv", [D_IN, PROJ_IN], F32R, isOutput=False)
    wp_d = nc.declare_dram_parameter("wp", [PROJ_IN, D_IN], BF16, isOutput=False)
    bqk_d = nc.declare_dram_parameter("bqk", [128, 4], F32, isOutput=False)
    bv_d = nc.declare_dram_parameter("bv", [128, H], F32, isOutput=False)
    b2_d = nc.declare_dram_parameter("b2", [128, 4], F32, isOutput=False)
    outT_d = nc.declare_dram_parameter("outT", [BPC, D_IN, S], F32, isOutput=True)
    import os as _os
    _DBG = bool(_os.environ.get("KDBG"))
    if _DBG:
        dbgM_d = nc.declare_dram_parameter("dbgM", [2, PROJ_IN + 64, S], mybir.dt.bfloat16, isOutput=True)
        dbgHT_d = nc.declare_dram_parameter("dbgHT", [128, 2, 208], mybir.dt.bfloat16, isOutput=True)
    _DBG2 = bool(_os.environ.get("KDBG2"))
    if _DBG2:
        dbgHT2_d = nc.declare_dram_parameter("dbgHT2", [8, 128, 2, 208], mybir.dt.bfloat16, isOutput=True)
        dbgOB_d = nc.declare_dram_parameter("dbgOB", [128, 4, S2], mybir.dt.float32, isOutput=True)
        dbgM2_d = nc.declare_dram_parameter("dbgM2", [2, PROJ_IN + 64, S], mybir.dt.bfloat16, isOutput=True)

    MT = [(0, 128), (128, 68)]  # seq m/k tile (offset, size)

    with tile.TileContext(nc) as tc, ExitStack() as ctx:
        wpool = ctx.enter_context(tc.tile_pool(name="w", bufs=1))
        xpool = ctx.enter_context(tc.tile_pool(name="x", bufs=2))
        qkpool = ctx.enter_context(tc.tile_pool(name="qk", bufs=2))
        vpool = ctx.enter_context(tc.tile_pool(name="v", bufs=2))
        epool = ctx.enter_context(tc.tile_pool(name="e", bufs=3))
        apool = ctx.enter_context(tc.tile_pool(name="a", bufs=3))
        hpool = ctx.enter_context(tc.tile_pool(name="h", bufs=2))
        opool = ctx.enter_context(tc.tile_pool(name="o", bufs=2))
        psum = ctx.enter_context(tc.tile_pool(name="ps", bufs=8, space="PSUM"))
        mdram = ctx.enter_context(tc.tile_pool(name="md", bufs=3, space="DRAM"))

        def load_xt(pair_idx):
            bb = 2 * pair_idx
            tiles = []
            for k in range(4):
                t = xpool.tile([128, 2, S], F32R, tag=f"xt{k}", bufs=2,
                               name=f"xt{pair_idx}_{k}")
                nc.sync.dma_start(
                    out=t,
                    in_=xT_d[bb:bb + 2, k * 128:(k + 1) * 128, :].rearrange(
                        "b p s -> p b s"),
                )
                tiles.append(t)
            return tiles

        # ---- persistent weights / constants ----
        # interleave the first pair's xT tiles with the Wqk tiles so the first
        # QK matmul's two dependencies are the first two DMAs in the queue
        wqk_sb = []
        wv_sb = []
        wp_sb = []
        xt_next = []
        for k in range(4):
            t = wpool.tile([128, 512], F32R, tag=f"wqk{k}")
            nc.sync.dma_start(out=t, in_=wqk_d[k * 128:(k + 1) * 128, :])
            wqk_sb.append(t)
            tx = xpool.tile([128, 2, S], F32R, tag=f"xt{k}", bufs=2,
                            name=f"xt0_{k}")
            nc.sync.dma_start(
                out=tx,
                in_=xT_d[0:2, k * 128:(k + 1) * 128, :].rearrange(
                    "b p s -> p b s"))
            xt_next.append(tx)
        for k in range(4):
            t = wpool.tile([128, PROJ_IN], F32R, tag=f"wv{k}")
            nc.sync.dma_start(out=t, in_=wv_d[k * 128:(k + 1) * 128, :])
            wv_sb.append(t)
        for k in range(8):
            t = wpool.tile([128, 512], BF16, tag=f"wp{k}")
            nc.sync.dma_start(out=t, in_=wp_d[k * 128:(k + 1) * 128, :])
            wp_sb.append(t)
        bqk_sb = wpool.tile([128, 4], F32, tag="bqk")
        nc.sync.dma_start(out=bqk_sb, in_=bqk_d[:, :])
        bv_sb = wpool.tile([128, H], F32, tag="bv")
        nc.sync.dma_start(out=bv_sb, in_=bv_d[:, :])
        b2_sb = wpool.tile([128, 4], F32, tag="b2")
        nc.sync.dma_start(out=b2_sb, in_=b2_d[:, :])

        ones_bf = wpool.tile([128, 1], BF16, tag="ones_bf")
        nc.vector.memset(ones_bf, 1.0)
        ident_f = wpool.tile([128, 128], F32, tag="ident_f")
        make_identity(nc, ident_f)
        ident = wpool.tile([128, 128], BF16, tag="ident")
        nc.vector.tensor_copy(ident, ident_f)

        pending_out = None  # deferred output stage of the previous pair

        def load_h_tiles(b0_, md_):
            """Emit H loads right after the avT writes: one full-width DMA per
            (bi, i-block) — 2KB contiguous per partition, minimal DMA count."""
            hs = []
            for bi in range(2):
                hview = md_[bi].rearrange("p s -> (p s)")[0:200704]                    .rearrange("(i j) -> i j", j=PROJ_IN)
                hb0 = hpool.tile([128, PROJ_IN], BF16, tag=f"hb0_{bi}",
                                 name=f"hb0_{b0_}_{bi}")
                nc.sync.dma_start(out=hb0, in_=hview[0:128, :])
                hb1 = hpool.tile([128, PROJ_IN], BF16, tag=f"hb1_{bi}",
                                 name=f"hb1_{b0_}_{bi}")
                nc.sync.dma_start(out=hb1[0:68], in_=hview[128:S, :])
                hs.append((hb0, hb1))
            return hs

        def emit_out_chunk(b0_, hs_, J2, ht_tiles):
            pts = [psum.tile([128, S2], BF16, tag="ps",
                             name=f"pt{b0_}_{J2}_{jj2}") for jj2 in range(2)]
            for bi in range(2):
                hb0, hb1 = hs_[bi]
                for jj in range(2):
                    c0 = J2 * 256 + jj * 128
                    nc.tensor.transpose(
                        pts[jj][:, bi * S:bi * S + 128],
                        hb0[:, c0:c0 + 128], ident)
                    nc.tensor.transpose(
                        pts[jj][:, bi * S + 128:bi * S + S],
                        hb1[0:68, c0:c0 + 128], ident[0:68, 0:68])
            for jj in range(2):
                ht = hpool.tile([128, S2], BF16, tag=f"ht{2 * J2 + jj}")
                nc.any.tensor_copy(ht, pts[jj])
                ht_tiles.append(ht)

        def run_output_stage(po_args, ht_tiles=None):
            (b0_, hs_) = po_args
            if ht_tiles is None:
                ht_tiles = []
            for J2 in range(4 - len(ht_tiles) // 2, 4):
                pass  # chunks already emitted by caller
            for J2 in range(len(ht_tiles) // 2, 4):
                emit_out_chunk(b0_, hs_, J2, ht_tiles)

            ob = opool.tile([128, 4, S2], F32, tag="ob", bufs=2,
                            name=f"ob{b0_}")
            for m in range(4):
                po = psum.tile([128, S2], F32, tag="ps")
                for kk in range(8):
                    nc.tensor.matmul(
                        po, wp_sb[kk][:, m * 128:(m + 1) * 128], ht_tiles[kk],
                        start=(kk == 0), stop=(kk == 7),
                    )
                nc.any.tensor_scalar_add(ob[:, m, :], po, b2_sb[:, m:m + 1])
            for bi in range(2):
                nc.sync.dma_start(
                    out=outT_d[b0_ + bi].rearrange("(m p) s -> p m s", p=128),
                    in_=ob[:, :, bi * S:(bi + 1) * S])
            if _DBG2 and b0_ == 0:
                for bi in range(2):
                    nc.sync.dma_start(out=dbgM2_d[bi], in_=md_[bi][:, :])
                nc.sync.dma_start(out=dbgOB_d[:, :, :], in_=ob[:, :, :])

        for pr in range(NPAIR):
            b0 = 2 * pr
            xt = xt_next
            if pr + 1 < NPAIR:
                xt_next = load_xt(pr + 1)

            # ---- qkT: rows 0:256 = q heads, 256:512 = k heads ----
            qk_sb = []
            for m in range(4):
                ps = psum.tile([128, S2], F32, tag="ps")
                for k in range(4):
                    nc.tensor.matmul(
                        ps, wqk_sb[k][:, m * 128:(m + 1) * 128], xt[k],
                        start=(k == 0), stop=(k == 3),
                    )
                qt = qkpool.tile([128, 2, S], BF16, tag=f"qk{m}", bufs=3)
                nc.any.tensor_scalar_add(qt, ps, bqk_sb[:, m:m + 1])
                qk_sb.append(qt)

            # ---- v natural per batch: [196, 1024] bf16, 2 seq tiles ----
            v_sb = [[None, None], [None, None]]
            for bi in range(2):
                for m2, (off, sz) in enumerate(MT):
                    vt = vpool.tile([128, PROJ_IN], BF16, tag=f"v{bi}{m2}", bufs=3)
                    for n in range(2):
                        ps = psum.tile([128, 512], F32, tag="ps")
                        for k in range(4):
                            nc.tensor.matmul(
                                ps[:sz], xt[k][:, bi, off:off + sz],
                                wv_sb[k][:, n * 512:(n + 1) * 512],
                                start=(k == 0), stop=(k == 3),
                            )
                        nc.any.tensor_copy(vt[:sz, n * 512:(n + 1) * 512], ps[:sz])
                    v_sb[bi][m2] = vt

            # ---- M scratch (one per batch): [1024, 196] = stacked avT ----
            md = [mdram.tile([PROJ_IN + 64, S], BF16, tag=f"M{bi}", name=f"md{pr}_{bi}") for bi in range(2)]

            # ---- attention, stage-grouped across heads for overlap ----
            # Phase A: S^T matmuls (pair psum, bi halves) + exp -> et tiles
            ets = []
            for h in range(8):
                g, hh = divmod(h, 4)
                q_tile, k_tile = qk_sb[g], qk_sb[2 + g]
                r0 = 32 * hh
                et = [epool.tile([128, 2, S], BF16, tag=f"et{h}_{mt}",
                                  name=f"et{pr}_{h}_{mt}", bufs=2)
                      for mt in range(2)]
                for mt, (off, sz) in enumerate(MT):
                    ps = psum.tile([128, S2], F32, tag="ps")
                    for bi in range(2):
                        nc.tensor.matmul(
                            ps[:sz, bi * S:(bi + 1) * S],
                            k_tile[r0:r0 + 32, bi, off:off + sz],
                            q_tile[r0:r0 + 32, bi, :],
                            start=True, stop=True,
                            tile_position=(r0, 0),
                        )
                    nc.scalar.activation(et[mt][:sz], ps[:sz], AFT.Exp,
                                         scale=SCALE)
                ets.append(et)

            # Phase B: colsums (PE ones-matmul) + reciprocals (DVE)
            rws_all = []
            for h in range(8):
                et = ets[h]
                pc = psum.tile([1, S2], F32, tag="ps")
                nc.tensor.matmul(pc, ones_bf, et[0], start=True, stop=False)
                nc.tensor.matmul(pc, ones_bf[0:68], et[1][0:68],
                                 start=False, stop=True)
                rws = apool.tile([1, S2], F32R, tag=f"rws{h % 2}", bufs=2,
                                 name=f"rws{pr}_{h}")
                with nc.allow_low_precision(reason="fp22 fine for softmax norm"):
                    nc.vector.reciprocal(rws, pc)
                rws_all.append(rws)

            # Phase C: all R broadcasts up-front so GPSIMD never gates PSUM drain
            R_all = []
            for h in range(8):
                R_h = apool.tile([128, S2], F32R, tag=f"R{h}", bufs=1,
                                 name=f"R{pr}_{h}")
                nc.gpsimd.partition_broadcast(R_h, rws_all[h][0:1, :])
                R_all.append(R_h)

            # Per head: U^T + hswish chain; M write merged below.
            # The previous pair's transpose chunks slot in every other head to
            # fill PE bubbles while the hswish chain drains U^T psum slots.
            out_ht_tiles = []
            av_all = apool.tile([128, 8, S2], BF16, tag="av_all", bufs=2,
                                name=f"av_all{pr}")
            def chain_tail(h, t_):
                # hswish tail: a = t + bv; avT = a*min(relu((a+3))/6,1)
                a_ = apool.tile([128, S2], F32, tag="a", bufs=3,
                                name=f"a{pr}_{h}")
                nc.vector.tensor_scalar_add(a_, t_, bv_sb[:, h:h + 1])
                u_ = apool.tile([128, S2], F32, tag="u", bufs=3,
                                name=f"u{pr}_{h}")
                nc.gpsimd.tensor_scalar(u_, a_, 3.0, 0.0, ALU.add, ALU.max)
                w_ = apool.tile([128, S2], F32, tag="wm", bufs=3,
                                name=f"w{pr}_{h}")
                nc.gpsimd.tensor_scalar(w_, u_, 6.0, 1.0 / 6.0, ALU.min,
                                        ALU.mult)
                nc.vector.tensor_mul(av_all[:, h, :], a_, w_)

            # Two waves of 4 heads: AV matmuls + t_ (frees the U^T psum fast)
            # first, hswish tails after — keeps PE's psum slots draining at
            # DVE t_ rate instead of the full cross-engine chain rate.
            t_held = {}
            for wave in range(2):
                for h in range(4 * wave, 4 * wave + 4):
                    if pending_out is not None and h >= 4 and h % 2 == 0:
                        emit_out_chunk(pending_out[0], pending_out[1],
                                       (h - 4) // 2, out_ht_tiles)
                    et = ets[h]
                    R_h = R_all[h]
                    pu = psum.tile([128, S2], F32, tag="ps")
                    for bi in range(2):
                        nc.tensor.matmul(
                            pu[:, bi * S:(bi + 1) * S],
                            v_sb[bi][0][:, h * 128:(h + 1) * 128],
                            et[0][:, bi, :], start=True, stop=False)
                        nc.tensor.matmul(
                            pu[:, bi * S:(bi + 1) * S],
                            v_sb[bi][1][0:68, h * 128:(h + 1) * 128],
                            et[1][0:68, bi, :], start=False, stop=True)
                    t_ = apool.tile([128, S2], F32, tag=f"t{h % 4}", bufs=2,
                                    name=f"t{pr}_{h}")
                    nc.vector.tensor_mul(t_, pu, R_h)
                    t_held[h] = t_
                for h in range(4 * wave, 4 * wave + 4):
                    chain_tail(h, t_held.pop(h))
            av_writes = []
            for bi in range(2):
                wr = nc.sync.dma_start(
                    out=md[bi][0:PROJ_IN, :].rearrange("(h p) s -> p h s", p=128),
                    in_=av_all[:, :, bi * S:(bi + 1) * S])
                av_writes.append(wr)

            if _DBG and pr == 0:
                for bi in range(2):
                    nc.sync.dma_start(out=dbgM_d[bi], in_=md[bi][:, :])

            hs_now = load_h_tiles(b0, md)

            # ---- output stage: finish the PREVIOUS pair's (chunks 2,3 + proj)
            if pending_out is not None:
                run_output_stage(pending_out, out_ht_tiles)
            pending_out = (b0, hs_now)

        run_output_stage(pending_out)

    nc.compile()
    return nc


def _get_nc():
    if "nc" not in _cache:
        _cache["nc"] = _build()
    return _cache["nc"]


def _prep(inputs):
    """Host-side BN folding / weight permutation / x transpose."""
    f = np.float32
    gamma1, beta1 = inputs["gamma1"].astype(f), inputs["beta1"].astype(f)
    mean1, var1 = inputs["mean1"].astype(f), inputs["var1"].astype(f)
    gamma2, beta2 = inputs["gamma2"].astype(f), inputs["beta2"].astype(f)
    mean2, var2 = inputs["mean2"].astype(f), inputs["var2"].astype(f)

    a1 = gamma1 / np.sqrt(var1 + EPS)
    c1 = beta1 - mean1 * a1
    W1 = inputs["W_qkv"].astype(f) * a1[None, :]
    B1 = inputs["b_qkv"].astype(f) * a1 + c1
    a2 = gamma2 / np.sqrt(var2 + EPS)
    c2 = beta2 - mean2 * a2
    W2 = np.ascontiguousarray(inputs["W_proj"].astype(f) * a2[None, :])
    B2 = inputs["b_proj"].astype(f) * a2 + c2

    W1h = W1.reshape(D_IN, H, 2 * DK + DV)
    B1h = B1.reshape(H, 2 * DK + DV)
    Wq = W1h[:, :, 0:DK].reshape(D_IN, H * DK)
    Wk = W1h[:, :, DK:2 * DK].reshape(D_IN, H * DK)
    Wv = np.ascontiguousarray(W1h[:, :, 2 * DK:].reshape(D_IN, PROJ_IN))
    Wqk = np.ascontiguousarray(np.concatenate([Wq, Wk], axis=1))
    bqk = np.concatenate([B1h[:, 0:DK].reshape(-1), B1h[:, DK:2 * DK].reshape(-1)])
    bv = B1h[:, 2 * DK:].reshape(-1)  # [1024] head-major

    x = inputs["x"].astype(f)
    xT = np.ascontiguousarray(x.transpose(0, 2, 1))  # [B, 512, 196]

    bqk_r = np.ascontiguousarray(bqk.reshape(4, 128).T)   # [128, 4]
    b2_r = np.ascontiguousarray(B2.reshape(4, 128).T)     # [128, 4]
    bv_r = np.ascontiguousarray(bv.reshape(H, DV).T)      # [128, 8]
    return xT, Wqk, Wv, W2, bqk_r, bv_r, b2_r


def kernel(**inputs) -> np.ndarray:
    import ml_dtypes
    from concourse.bass_utils import run_bass_kernel_spmd

    xT, Wqk, Wv, W2, bqk_r, bv_r, b2_r = _prep(inputs)
    nc = _get_nc()

    W2b = W2.astype(ml_dtypes.bfloat16)
    in_maps = []
    for c in range(NCORES):
        in_maps.append({
            "xT": np.ascontiguousarray(xT[c * BPC:(c + 1) * BPC]),
            "wqk": Wqk, "wv": Wv, "wp": W2b,
            "bqk": bqk_r, "bv": bv_r, "b2": b2_r,
        })
    res = run_bass_kernel_spmd(nc, in_maps, list(range(NCORES)))
    outT = np.concatenate([res.results[c]["outT"] for c in range(NCORES)], axis=0)
    out = np.ascontiguousarray(outT.transpose(0, 2, 1)).astype(np.float32)
    return out
